# revision 1
# baseline (speedup 1.0000x reference)
"""Trainium2 Bass kernel for nn_CorrelationRegularizer.

Reference computation (see problem statement):
    mean/std per column over the batch (population std),
    fs = (w - mean) / std
    s[b]  = sum_n fs[b, n]
    s2[b] = sum_n fs[b, n]^2
    pair_mean[b] = (s^2 - s2) / (N * (N - 1))
    out = L * mean_b |pair_mean[b]|

Strategy (8 NeuronCores, data-parallel over the batch axis):
  * Each core holds a [2048, 2048] fp32 shard of w, SBUF-resident
    (one HBM read total).
  * Phase A (overlapped with the input DMA): per-column sums of w and w^2
    via TensorE ones-matmuls (column sums are partition-axis reductions);
    w^2 tiles produced on ScalarE.
  * A 16 KB AllReduce combines the per-core column sums -> global
    mean/std -> per-column weights a = 1/sigma and c2 = 2*a^2*mean.
  * Phase B: with t = a (.) w (per-column scale),
        s_raw[b] = sum_n a_n w[b,n]              (DVE tensor_tensor_reduce)
        r2[b]    = sum_n c2_n w[b,n]             (DVE tensor_tensor_reduce)
        q[b]     = sum_n t[b,n]^2                (ScalarE Square + accum_out)
    and with K = sum_n a_n mean_n, C2 = sum_n (a_n mean_n)^2:
        num[b] = (s_raw - K)^2 - q + r2 - C2  ==  (s^2 - s2)   [exact algebra]
  * Per-core output = sum_b |num[b]|; the host sums the 8 scalars and
    applies L / (B * N * (N-1)).
"""

import numpy as np

import concourse.bass as bass
import concourse.bacc as bacc
import concourse.mybir as mybir
import concourse.tile as tile
from concourse import bass_utils

F32 = mybir.dt.float32
AF = mybir.ActivationFunctionType
ALU = mybir.AluOpType
AX = mybir.AxisListType

B = 16384          # full batch
N = 2048           # columns (units)
L = 1.0            # regularizer scale
NCORES = 8
BSH = B // NCORES  # batch rows per core = 2048
P = 128            # partitions
NT = BSH // P      # batch tiles per core = 16
NDMA = 4           # input DMA transfers (4 tiles each)


def _build_kernel():
    nc = bacc.Bacc(
        "TRN2",
        target_bir_lowering=False,
        debug=False,
        enable_asserts=False,
        num_devices=NCORES,
    )
    w_ext = nc.dram_tensor("w", [BSH, N], F32, kind="ExternalInput")
    out_ext = nc.dram_tensor("out", [1, 1], F32, kind="ExternalOutput")
    cc_in = nc.dram_tensor("cc_in", [2, N], F32)
    cc_out = nc.dram_tensor("cc_out", [2, N], F32, addr_space="Shared")

    with tile.TileContext(nc) as tc:
        _body(nc, tc, w_ext, out_ext, cc_in, cc_out)
    nc.compile()
    return nc


def _body(nc, tc, w_ext, out_ext, cc_in, cc_out):
    invB = 1.0 / float(B)
    with (
        tc.tile_pool(name="const", bufs=1) as constp,
        tc.tile_pool(name="res", bufs=1) as resp,
        tc.tile_pool(name="sqp", bufs=2) as sqp,
        tc.tile_pool(name="stat", bufs=1) as statp,
        tc.tile_pool(name="rowp", bufs=1) as rowp,
    ):
        ones = constp.tile([P, P], F32)
        nc.vector.memset(ones[:, :], 1.0)

        # ---- resident input: [128, 16*2048], tile i = wres[:, i*N:(i+1)*N]
        wres = resp.tile([P, NT * N], F32)
        wres3 = wres[:, :].rearrange("p (i n) -> p i n", i=NT)
        w_r = w_ext.ap().rearrange("(i p) n -> p i n", p=P)
        per = NT // NDMA
        for g in range(NDMA):
            nc.sync.dma_start(
                out=wres3[:, g * per:(g + 1) * per, :],
                in_=w_r[:, g * per:(g + 1) * per, :],
            )

        # ---- phase A: column sums of w and w^2 (partition-axis via PE)
        with tc.tile_pool(name="psA", bufs=1, space="PSUM") as psA:
            ps_s = psA.tile([1, N], F32)
            ps_q = psA.tile([1, N], F32)
            for i in range(NT):
                w_i = wres[:, i * N:(i + 1) * N]
                sq = sqp.tile([P, N], F32, tag="sq")
                nc.scalar.activation(sq[:, :], w_i, AF.Square)
                for c in range(4):
                    sl = slice(c * 512, (c + 1) * 512)
                    nc.tensor.matmul(
                        ps_s[0:1, sl], lhsT=ones[:, 0:1], rhs=w_i[:, sl],
                        start=(i == 0), stop=(i == NT - 1),
                    )
                    nc.tensor.matmul(
                        ps_q[0:1, sl], lhsT=ones[:, 0:1], rhs=sq[:, sl],
                        start=(i == 0), stop=(i == NT - 1),
                    )
            s1row = rowp.tile([1, N], F32, tag="row0")
            s2row = rowp.tile([1, N], F32, tag="row1")
            nc.scalar.copy(s1row[0:1, :], ps_s[0:1, :])
            nc.vector.tensor_copy(s2row[0:1, :], ps_q[0:1, :])

        # ---- all-reduce the local column sums across the 8 cores
        nc.sync.dma_start(out=cc_in.ap()[0:1, :], in_=s1row[0:1, :])
        nc.sync.dma_start(out=cc_in.ap()[1:2, :], in_=s2row[0:1, :])
        nc.gpsimd.collective_compute(
            "AllReduce",
            ALU.add,
            replica_groups=[list(range(NCORES))],
            ins=[cc_in.ap()],
            outs=[cc_out.ap()],
        )

        # ---- stats in [16, 128] layout (n = 128*g + f)
        S1 = statp.tile([16, P], F32)
        S2 = statp.tile([16, P], F32)
        nc.sync.dma_start(out=S1[:, :], in_=cc_out.ap()[0, :].rearrange("(g f) -> g f", g=16))
        nc.sync.dma_start(out=S2[:, :], in_=cc_out.ap()[1, :].rearrange("(g f) -> g f", g=16))

        mean = statp.tile([16, P], F32)
        nc.scalar.activation(mean[:, :], S1[:, :], AF.Copy, 0.0, invB)
        ex2 = statp.tile([16, P], F32)
        nc.scalar.activation(ex2[:, :], S2[:, :], AF.Copy, 0.0, invB)
        msq = statp.tile([16, P], F32)
        nc.scalar.activation(msq[:, :], mean[:, :], AF.Square)
        var = statp.tile([16, P], F32)
        nc.vector.tensor_tensor(var[:, :], ex2[:, :], msq[:, :], ALU.subtract)
        u = statp.tile([16, P], F32)
        nc.vector.reciprocal(u[:, :], var[:, :])          # u = 1/var = a^2
        a16 = statp.tile([16, P], F32)
        nc.scalar.activation(a16[:, :], u[:, :], AF.Sqrt)  # a = 1/sigma
        c2v = statp.tile([16, P], F32)                     # c2 = 2*a^2*mean
        nc.vector.scalar_tensor_tensor(c2v[:, :], u[:, :], 2.0, mean[:, :], ALU.mult, ALU.mult)
        v1 = statp.tile([16, P], F32)                      # v = a*mean
        nc.vector.tensor_tensor(v1[:, :], a16[:, :], mean[:, :], ALU.mult)
        vsq = statp.tile([16, P], F32)
        nc.scalar.activation(vsq[:, :], v1[:, :], AF.Square)
        kk = statp.tile([16, 2], F32)                      # K, C2 partials
        nc.vector.reduce_sum(kk[:, 0:1], v1[:, :], axis=AX.X)
        nc.vector.reduce_sum(kk[:, 1:2], vsq[:, :], axis=AX.X)

        # rows at partition 0 for the PE broadcast (moving operand)
        a_row = rowp.tile([1, N], F32, tag="row0")
        c2_row = rowp.tile([1, N], F32, tag="row1")
        nc.sync.dma_start(out=a_row[0:1, :], in_=a16[:, :])
        nc.sync.dma_start(out=c2_row[0:1, :], in_=c2v[:, :])

        # ---- broadcast a, c2 down all 128 partitions; K, C2 to [128, 2]
        a_bc = resp.tile([P, N], F32)
        c2_bc = resp.tile([P, N], F32)
        KC = statp.tile([P, 2], F32)
        with tc.tile_pool(name="psB", bufs=2, space="PSUM") as psB:
            ps_kc = psB.tile([P, 2], F32, tag="kc")
            nc.tensor.matmul(ps_kc[:, :], lhsT=ones[0:16, :], rhs=kk[:, :], start=True, stop=True)
            nc.scalar.copy(KC[:, :], ps_kc[:, :])
            for row, dst in ((a_row, a_bc), (c2_row, c2_bc)):
                for c in range(4):
                    ps_b = psB.tile([P, 512], F32, tag="bc")
                    for k in range(4):
                        col = c * 512 + k * 128
                        nc.tensor.matmul(
                            ps_b[:, k * 128:(k + 1) * 128],
                            lhsT=ones[0:1, :],
                            rhs=row[0:1, col:col + 128],
                            start=True, stop=True,
                        )
                    nc.vector.tensor_copy(dst[:, c * 512:(c + 1) * 512], ps_b[:, :])

        # ---- phase B: per-row reductions
        s_all = statp.tile([P, NT], F32)
        q_all = statp.tile([P, NT], F32)
        r_all = statp.tile([P, NT], F32)
        with tc.tile_pool(name="tp", bufs=2) as tp:
            for i in range(NT):
                w_i = wres[:, i * N:(i + 1) * N]
                t = tp.tile([P, N], F32, tag="t")
                nc.vector.tensor_tensor(t[:, :], w_i, a_bc[:, :], ALU.mult)
                nc.vector.reduce_sum(s_all[:, i:i + 1], t[:, :], axis=AX.X)
                r_t = sqp.tile([P, N], F32, tag="sq")
                nc.vector.tensor_tensor(r_t[:, :], w_i, c2_bc[:, :], ALU.mult)
                nc.vector.reduce_sum(r_all[:, i:i + 1], r_t[:, :], axis=AX.X)
                # square pass: output overwrites w_i (dead after this tile)
                nc.scalar.activation(w_i, t[:, :], AF.Square)
                nc.vector.reduce_sum(q_all[:, i:i + 1], w_i, axis=AX.X)

        # ---- final: num = (s_raw - K)^2 - q + r2 - C2; out = sum_b |num|
        negKC = statp.tile([P, 2], F32)
        nc.scalar.mul(negKC[:, :], KC[:, :], -1.0)
        sf = statp.tile([P, NT], F32)
        nc.scalar.activation(sf[:, :], s_all[:, :], AF.Identity, negKC[:, 0:1])
        ss = statp.tile([P, NT], F32)
        nc.scalar.activation(ss[:, :], sf[:, :], AF.Square)
        d1 = statp.tile([P, NT], F32)
        nc.vector.tensor_tensor(d1[:, :], ss[:, :], q_all[:, :], ALU.subtract)
        d2 = statp.tile([P, NT], F32)
        nc.vector.tensor_tensor(d2[:, :], d1[:, :], r_all[:, :], ALU.add)
        num = statp.tile([P, NT], F32)
        nc.scalar.activation(num[:, :], d2[:, :], AF.Identity, negKC[:, 1:2])
        ab = statp.tile([P, NT], F32)
        nc.scalar.activation(ab[:, :], num[:, :], AF.Abs)
        tot = statp.tile([P, 1], F32)
        nc.vector.reduce_sum(tot[:, :], ab[:, :], axis=AX.X)
        res_sb = statp.tile([1, 1], F32)
        with tc.tile_pool(name="psF", bufs=1, space="PSUM") as psF:
            ps_f = psF.tile([1, 1], F32)
            nc.tensor.matmul(ps_f[0:1, 0:1], lhsT=tot[:, :], rhs=ones[:, 0:1], start=True, stop=True)
            nc.scalar.copy(res_sb[0:1, :], ps_f[0:1, :])
        nc.sync.dma_start(out=out_ext.ap(), in_=res_sb[0:1, :])


_CACHE = {}
LAST_RESULTS = None


def _get_nc():
    if "nc" not in _CACHE:
        _CACHE["nc"] = _build_kernel()
    return _CACHE["nc"]


def kernel(w):
    global LAST_RESULTS
    w = np.asarray(w, dtype=np.float32)
    assert w.shape == (B, N), f"unexpected input shape {w.shape}"
    nc = _get_nc()
    in_maps = [
        {"w": np.ascontiguousarray(w[c * BSH:(c + 1) * BSH])} for c in range(NCORES)
    ]
    res = bass_utils.run_bass_kernel_spmd(nc, in_maps, core_ids=list(range(NCORES)))
    LAST_RESULTS = res
    total = sum(float(r["out"][0, 0]) for r in res.results)
    val = L * total / (float(B) * float(N) * float(N - 1))
    return np.float32(val)

